# revision 2
# baseline (speedup 1.0000x reference)
"""DenseCL contrastive loss on 8 Trainium2 NeuronCores (Bass/Tile).

Strategy: data-parallel over batch B=128 -> 16 batches/core.
Per core (T-layouts, contraction dims on partitions):
  - dense head per branch:  H1T = W1^T @ X^T (bf16), ZT = W2^T @ H1T,
    l2norm over De via gpsimd partition_all_reduce -> f1T/f2T [128, 784]
  - global head: gT = mean_HW(X^T) then same 2-layer MLP -> qgT/kgT [128,16]
  - sim_b = f1T_b^T @ f2T_b -> [49,49]; DVE max/max_index -> pos + argmax
  - matchedT gathered from f2T columns via gpsimd ap_gather
  - AllGather of kgT (early) and matchedT (after argmax) across 8 cores
  - row-block of InfoNCE logits: f1T^T @ matchedT_all, exp on ACT with
    accumulated row sums, log -> lse; loss partials reduced to scalars
Host sums 4 partial scalars per core into the final loss.
"""

import sys

sys.path.insert(0, "/opt/trn_rl_repo")

import numpy as np
import ml_dtypes

import concourse.bacc as bacc
import concourse.mybir as mybir
import concourse.bass_isa as bass_isa
import concourse.tile as tile
from concourse.bass_utils import run_bass_kernel_spmd

dt = mybir.dt
AF = mybir.ActivationFunctionType

N_CORES = 8
B, H, W, C = 128, 7, 7, 2048
DH, DE = 2048, 128
HW = H * W                      # 49
BL = B // N_CORES               # 16 batches per core
PIX = BL * HW                   # 784 pixels per core
GPIX = B * HW                   # 6272 global rows
TAU_INV = 5.0
KC = C // 128                   # 16 contraction chunks
MC = DH // 128                  # 16 hidden chunks

_NC = None


def _build():
    nc = bacc.Bacc("TRN2", target_bir_lowering=False, debug=False,
                   num_devices=N_CORES)

    def inp(name, shape, d=dt.bfloat16):
        return nc.dram_tensor(name, shape, d, kind="ExternalInput").ap()

    xqT = inp("xqT", [C, PIX])
    xkT = inp("xkT", [C, PIX])
    wd1 = inp("wd1", [C, DH])
    wg1 = inp("wg1", [C, DH])
    wmd1 = inp("wmd1", [C, DH])
    wmg1 = inp("wmg1", [C, DH])
    wd2 = inp("wd2", [DH, DE])
    wg2 = inp("wg2", [DH, DE])
    wmd2 = inp("wmd2", [DH, DE])
    wmg2 = inp("wmg2", [DH, DE])
    bd1 = inp("bd1", [128, MC], dt.float32)
    bg1 = inp("bg1", [128, MC], dt.float32)
    mbd1 = inp("mbd1", [128, MC], dt.float32)
    mbg1 = inp("mbg1", [128, MC], dt.float32)
    bd2 = inp("bd2", [128, 1], dt.float32)
    bg2 = inp("bg2", [128, 1], dt.float32)
    mbd2 = inp("mbd2", [128, 1], dt.float32)
    mbg2 = inp("mbg2", [128, 1], dt.float32)
    eye = inp("eye", [64, 64], dt.float32)
    addv = inp("addv", [BL, 1], dt.float32)
    out = nc.dram_tensor("partials", [1, 8], dt.float32,
                         kind="ExternalOutput").ap()

    with tile.TileContext(nc) as tc:
        with (
            tc.tile_pool(name="pers", bufs=1) as pers,
            tc.tile_pool(name="wz", bufs=2) as wz,
            tc.tile_pool(name="work", bufs=2) as work,
            tc.tile_pool(name="ps", bufs=2, space="PSUM") as ps,
            tc.tile_pool(name="dram", bufs=1, space="DRAM") as dram,
        ):
            # ---- constants / biases ----
            eyesb = pers.tile([64, 64], dt.float32, name="eyesb")
            nc.sync.dma_start(out=eyesb[:], in_=eye[:])
            addsb = pers.tile([BL, 1], dt.float32, name="addsb")
            nc.sync.dma_start(out=addsb[:], in_=addv[:])
            biases = {}
            for nm, src in (("bd1", bd1), ("bg1", bg1), ("mbd1", mbd1),
                            ("mbg1", mbg1)):
                t = pers.tile([128, MC], dt.float32, name=f"b_{nm}")
                nc.sync.dma_start(out=t[:], in_=src[:])
                biases[nm] = t
            for nm, src in (("bd2", bd2), ("bg2", bg2), ("mbd2", mbd2),
                            ("mbg2", mbg2)):
                t = pers.tile([128, 1], dt.float32, name=f"b_{nm}")
                nc.sync.dma_start(out=t[:], in_=src[:])
                biases[nm] = t

            def load_xt(x_dram, nm):
                ts = []
                for k in range(KC):
                    t = pers.tile([128, PIX], dt.bfloat16, name=f"{nm}{k}")
                    nc.sync.dma_start(
                        out=t[:], in_=x_dram[k * 128:(k + 1) * 128, :])
                    ts.append(t)
                return ts

            def norm_cols(z, n, nm, outs):
                """l2-normalize columns of z [128, n] (De on partitions).
                outs: list of (tile, dtype slice ap) to write z * rsqrt."""
                sq = work.tile([128, n], dt.float32, tag=f"sq{n}", name=f"sq_{nm}")
                nc.vector.tensor_mul(sq[:], z[:], z[:])
                ssr = work.tile([128, n], dt.float32, tag=f"ssr{n}", name=f"ssr_{nm}")
                nc.gpsimd.partition_all_reduce(ssr[:], sq[:], 128,
                                               bass_isa.ReduceOp.add)
                nc.vector.tensor_scalar_max(ssr[:], ssr[:], 1e-12)
                srt = work.tile([128, n], dt.float32, tag=f"srt{n}", name=f"srt_{nm}")
                nc.scalar.activation(srt[:], ssr[:], AF.Sqrt)
                rr = work.tile([128, n], dt.float32, tag=f"rr{n}", name=f"rr_{nm}")
                nc.vector.reciprocal(rr[:], srt[:])
                for o in outs:
                    nc.vector.tensor_mul(o, z[:], rr[:])

            def branch(xts, w1_dram, b1, w2_dram, b2, gw1_dram, gb1,
                       gw2_dram, gb2, nm):
                """Dense + global heads for one branch.
                Returns (zt [128,784] fp32, zg [128,16] fp32) un-normalized."""
                # mean over HW -> gT [128, 16] per c-chunk -> [128, 256] bf16
                gsum = work.tile([128, BL * KC], dt.float32, tag="gsum",
                                 name=f"gsum_{nm}")
                for k in range(KC):
                    nc.vector.tensor_reduce(
                        gsum[:, k * BL:(k + 1) * BL],
                        xts[k][:].rearrange("p (b w) -> p b w", w=HW),
                        axis=mybir.AxisListType.X, op=mybir.AluOpType.add)
                gt = work.tile([128, BL * KC], dt.bfloat16, tag="gt",
                               name=f"gt_{nm}")
                nc.vector.tensor_scalar_mul(gt[:], gsum[:], 1.0 / HW)

                w2sb = wz.tile([128, MC * 128], dt.bfloat16, tag="w2sb",
                               name=f"w2_{nm}")
                nc.sync.dma_start(
                    out=w2sb[:].rearrange("p (k m) -> p k m", m=DE),
                    in_=w2_dram[:].rearrange("(k p) m -> p k m", p=128))
                gw2sb = wz.tile([128, MC * 128], dt.bfloat16, tag="gw2sb",
                                name=f"gw2_{nm}")
                nc.sync.dma_start(
                    out=gw2sb[:].rearrange("p (k m) -> p k m", m=DE),
                    in_=gw2_dram[:].rearrange("(k p) m -> p k m", p=128))

                ztp = ps.tile([128, PIX], dt.float32, tag="ztp", bufs=1,
                              name=f"ztp_{nm}")
                hgt = work.tile([128, BL * MC], dt.bfloat16, tag="hgt",
                                name=f"hgt_{nm}")
                for m in range(MC):
                    # dense layer 1, hidden-chunk m
                    wcol = wz.tile([128, KC * 128], dt.bfloat16, tag="wcold",
                                   name=f"wcold_{nm}{m}")
                    nc.sync.dma_start(
                        out=wcol[:].rearrange("p (k m) -> p k m", m=128),
                        in_=w1_dram[:, m * 128:(m + 1) * 128].rearrange(
                            "(k p) m -> p k m", p=128))
                    h1p = ps.tile([128, PIX], dt.float32, tag="bigp",
                                  name=f"h1p_{nm}{m}")
                    for k in range(KC):
                        lhs = wcol[:, k * 128:(k + 1) * 128]
                        nc.tensor.matmul(h1p[:, 0:512], lhs, xts[k][:, 0:512],
                                         start=(k == 0), stop=(k == KC - 1))
                        nc.tensor.matmul(h1p[:, 512:PIX], lhs,
                                         xts[k][:, 512:PIX],
                                         start=(k == 0), stop=(k == KC - 1))
                    h1sb = work.tile([128, PIX], dt.bfloat16, tag="h1sb",
                                     name=f"h1_{nm}{m}")
                    nc.scalar.activation(h1sb[:], h1p[:], AF.Relu,
                                         bias=b1[:, m:m + 1])
                    lhs2 = w2sb[:, m * 128:(m + 1) * 128]
                    nc.tensor.matmul(ztp[:, 0:512], lhs2, h1sb[:, 0:512],
                                     start=(m == 0), stop=(m == MC - 1))
                    nc.tensor.matmul(ztp[:, 512:PIX], lhs2, h1sb[:, 512:PIX],
                                     start=(m == 0), stop=(m == MC - 1))

                    # global layer 1, hidden-chunk m
                    gwcol = wz.tile([128, KC * 128], dt.bfloat16, tag="wcolg",
                                    name=f"wcolg_{nm}{m}")
                    nc.sync.dma_start(
                        out=gwcol[:].rearrange("p (k m) -> p k m", m=128),
                        in_=gw1_dram[:, m * 128:(m + 1) * 128].rearrange(
                            "(k p) m -> p k m", p=128))
                    hp = ps.tile([128, BL], dt.float32, tag="smallp",
                                 name=f"hp_{nm}{m}")
                    for k in range(KC):
                        nc.tensor.matmul(hp[:], gwcol[:, k * 128:(k + 1) * 128],
                                         gt[:, k * BL:(k + 1) * BL],
                                         start=(k == 0), stop=(k == KC - 1))
                    nc.scalar.activation(hgt[:, m * BL:(m + 1) * BL], hp[:],
                                         AF.Relu, bias=gb1[:, m:m + 1])

                zt = work.tile([128, PIX], dt.float32, tag="zt",
                               name=f"zt_{nm}")
                nc.vector.tensor_scalar_add(zt[:], ztp[:], b2[:])

                zgp = ps.tile([128, BL], dt.float32, tag="smallp", name=f"zgp_{nm}")
                for m in range(MC):
                    nc.tensor.matmul(zgp[:], gw2sb[:, m * 128:(m + 1) * 128],
                                     hgt[:, m * BL:(m + 1) * BL],
                                     start=(m == 0), stop=(m == MC - 1))
                zg = work.tile([128, BL], dt.float32, tag="zg", name=f"zg_{nm}")
                nc.vector.tensor_scalar_add(zg[:], zgp[:], gb2[:])
                return zt, zg

            # ======== momentum (key) branch ========
            xkts = load_xt(xkT, "xk")
            ztk, zgk = branch(xkts, wmd1, biases["mbd1"], wmd2, biases["mbd2"],
                              wmg1, biases["mbg1"], wmg2, biases["mbg2"], "k")
            f2tb = pers.tile([128, PIX], dt.bfloat16, name="f2tb")
            f2tf = pers.tile([128, PIX], dt.float32, name="f2tf")
            norm_cols(ztk, PIX, "f2", [f2tb[:], f2tf[:]])
            kgtb = pers.tile([128, BL], dt.bfloat16, name="kgtb")
            kgtf = pers.tile([128, BL], dt.float32, name="kgtf")
            norm_cols(zgk, BL, "kg", [kgtb[:], kgtf[:]])

            # early AllGather of global keys
            ag2in = dram.tile([128, BL], dt.bfloat16, name="ag2in")
            ag2out = dram.tile([128 * N_CORES, BL], dt.bfloat16,
                               addr_space="Shared", name="ag2out")
            nc.sync.dma_start(out=ag2in[:], in_=kgtb[:])
            nc.gpsimd.collective_compute(
                "AllGather", mybir.AluOpType.bypass,
                replica_groups=[list(range(N_CORES))],
                ins=[ag2in.opt()], outs=[ag2out.opt()])

            # ======== query branch ========
            xqts = load_xt(xqT, "xq")
            ztq, zgq = branch(xqts, wd1, biases["bd1"], wd2, biases["bd2"],
                              wg1, biases["bg1"], wg2, biases["bg2"], "q")
            f1tb = pers.tile([128, PIX], dt.bfloat16, name="f1tb")
            norm_cols(ztq, PIX, "f1", [f1tb[:]])
            qgtb = pers.tile([128, BL], dt.bfloat16, name="qgtb")
            qgtf = pers.tile([128, BL], dt.float32, name="qgtf")
            norm_cols(zgq, BL, "qg", [qgtb[:], qgtf[:]])

            # ======== per-batch sim, argmax ========
            maxv = pers.tile([64, BL], dt.float32, name="maxv")
            nc.vector.memset(maxv[:], 0.0)
            idxc = pers.tile([64, BL], dt.float32, name="idxc")
            for b in range(BL):
                simp = ps.tile([64, HW], dt.float32, tag="smallp",
                               name=f"simp{b}")
                nc.tensor.matmul(simp[0:HW, :],
                                 f1tb[:, b * HW:(b + 1) * HW],
                                 f2tb[:, b * HW:(b + 1) * HW],
                                 start=True, stop=True)
                simsb = work.tile([64, HW], dt.float32, tag="simsb",
                                  name=f"sims{b}")
                nc.vector.tensor_copy(simsb[0:HW, :], simp[0:HW, :])
                mx8 = work.tile([64, 8], dt.float32, tag="mx8", name=f"mx{b}")
                mi8 = work.tile([64, 8], dt.uint16, tag="mi8", name=f"mi{b}")
                nc.vector.max(mx8[0:HW, :], simsb[0:HW, :])
                nc.vector.max_index(mi8[0:HW, :], mx8[0:HW, :], simsb[0:HW, :])
                nc.vector.tensor_copy(maxv[0:HW, b:b + 1], mx8[0:HW, 0:1])
                nc.vector.tensor_copy(idxc[0:HW, b:b + 1], mi8[0:HW, 0:1])

            # ---- wrapped gather indices [16, 49] ----
            tpp = ps.tile([BL, 64], dt.float32, tag="smallp", name="tpp")
            nc.tensor.transpose(tpp[0:BL, 0:HW], idxc[0:HW, 0:BL],
                                eyesb[0:HW, 0:HW])
            idxf = work.tile([BL, HW], dt.float32, tag="idxf", name="idxf")
            nc.vector.tensor_scalar_add(idxf[:], tpp[0:BL, 0:HW], addsb[:])
            idxw = work.tile([BL, HW], dt.int16, tag="idxw", name="idxw")
            nc.vector.tensor_copy(idxw[:], idxf[:])
            idxr = pers.tile([128, HW], dt.int16, name="idxr")
            for g in range(8):
                nc.sync.dma_start(out=idxr[g * 16:(g + 1) * 16, :],
                                  in_=idxw[:])

            # ---- gather matched keys, AllGather ----
            mtf = pers.tile([128, PIX], dt.float32, name="mtf")
            nc.gpsimd.ap_gather(mtf[:], f2tf[:], idxr[:], channels=128,
                                num_elems=PIX, d=1, num_idxs=PIX)
            mtb = pers.tile([128, PIX], dt.bfloat16, name="mtb")
            nc.vector.tensor_copy(mtb[:], mtf[:])
            ag1in = dram.tile([128, PIX], dt.bfloat16, name="ag1in")
            ag1out = dram.tile([128 * N_CORES, PIX], dt.bfloat16,
                               addr_space="Shared", name="ag1out")
            nc.sync.dma_start(out=ag1in[:], in_=mtb[:])
            nc.gpsimd.collective_compute(
                "AllGather", mybir.AluOpType.bypass,
                replica_groups=[list(range(N_CORES))],
                ins=[ag1in.opt()], outs=[ag1out.opt()])

            rhs = pers.tile([128, GPIX], dt.bfloat16, name="rhs")
            for r in range(N_CORES):
                nc.sync.dma_start(
                    out=rhs[:, r * PIX:(r + 1) * PIX],
                    in_=ag1out[r * 128:(r + 1) * 128, :])
            kgall = pers.tile([128, B], dt.bfloat16, name="kgall")
            for r in range(N_CORES):
                nc.sync.dma_start(
                    out=kgall[:, r * BL:(r + 1) * BL],
                    in_=ag2out[r * 128:(r + 1) * 128, :])

            # ======== dense InfoNCE row block ========
            chunks = []
            c0 = 0
            while c0 < GPIX:
                csz = min(1024, GPIX - c0)
                chunks.append((c0, csz))
                c0 += csz
            ncat = len(chunks)  # 7

            rsums = pers.tile([128, 8], dt.float32, name="rsums")
            nc.vector.memset(rsums[:], 1.0)
            for t in range(7):
                m0 = t * 128
                m = min(128, PIX - m0)
                rs = work.tile([128, 8], dt.float32, tag="rs", name=f"rs{t}")
                for ci, (c0, csz) in enumerate(chunks):
                    lp = ps.tile([128, 1024], dt.float32, tag="bigp",
                                 name=f"lp{t}_{ci}")
                    for h0 in range(0, csz, 512):
                        hsz = min(512, csz - h0)
                        nc.tensor.matmul(
                            lp[0:m, h0:h0 + hsz],
                            f1tb[:, m0:m0 + m],
                            rhs[:, c0 + h0:c0 + h0 + hsz],
                            start=True, stop=True)
                    expsb = work.tile([128, 1024], dt.bfloat16, tag="expsb",
                                      name=f"ex{t}_{ci}")
                    nc.scalar.activation(expsb[0:m, 0:csz], lp[0:m, 0:csz],
                                         AF.Exp, scale=TAU_INV,
                                         accum_out=rs[0:m, ci:ci + 1])
                nc.vector.tensor_reduce(rsums[0:m, t:t + 1],
                                        rs[0:m, 0:ncat],
                                        axis=mybir.AxisListType.X,
                                        op=mybir.AluOpType.add)
            lse = pers.tile([128, 8], dt.float32, name="lse")
            nc.scalar.activation(lse[:, 0:7], rsums[:, 0:7], AF.Ln)

            # ======== global InfoNCE ========
            lg = ps.tile([BL, B], dt.float32, tag="smallp", name="lg")
            nc.tensor.matmul(lg[0:BL, :], qgtb[:, 0:BL], kgall[:, 0:B],
                             start=True, stop=True)
            expg = work.tile([BL, B], dt.bfloat16, tag="expg", name="expg")
            eg = work.tile([BL, 1], dt.float32, tag="eg", name="eg")
            nc.scalar.activation(expg[0:BL, :], lg[0:BL, :], AF.Exp,
                                 scale=TAU_INV, accum_out=eg[0:BL, 0:1])
            lseg = work.tile([BL, 1], dt.float32, tag="lseg", name="lseg")
            nc.scalar.activation(lseg[:], eg[:], AF.Ln)
            pq = work.tile([128, BL], dt.float32, tag="pq", name="pq")
            nc.vector.tensor_mul(pq[:], qgtf[:], kgtf[:])

            # ======== final partial sums ========
            partials = pers.tile([1, 8], dt.float32, name="partials_sb")

            def psum_scalar(src, n_part, n_free, col, tagn):
                red = work.tile([n_part, 1], dt.float32, tag=f"red{tagn}",
                                name=f"red{tagn}")
                if n_free > 1:
                    nc.vector.tensor_reduce(red[:], src,
                                            axis=mybir.AxisListType.X,
                                            op=mybir.AluOpType.add)
                else:
                    nc.vector.tensor_copy(red[:], src)
                alr = work.tile([n_part, 1], dt.float32, tag=f"alr{tagn}",
                                name=f"alr{tagn}")
                nc.gpsimd.partition_all_reduce(alr[:], red[:], n_part,
                                               bass_isa.ReduceOp.add)
                nc.vector.tensor_copy(partials[0:1, col:col + 1],
                                      alr[0:1, 0:1])

            psum_scalar(lse[:, 0:7], 128, 7, 0, "a")     # sum lse_dense
            psum_scalar(maxv[:, 0:BL], 64, BL, 1, "b")   # sum max sim (pos_d/5)
            psum_scalar(lseg[:, 0:1], BL, 1, 2, "c")     # sum lse_global
            psum_scalar(pq[:, 0:BL], 128, BL, 3, "d")    # sum qg.kg (pos_g/5)

            nc.sync.dma_start(out=out[:], in_=partials[:])

    nc.compile()
    return nc


def _get_nc():
    global _NC
    if _NC is None:
        _NC = _build()
    return _NC


def _prep_inputs(inputs):
    bf = ml_dtypes.bfloat16
    f32 = np.float32
    w = {k: np.ascontiguousarray(inputs[k].astype(bf))
         for k in ("Wd1", "Wg1", "Wd2", "Wg2", "mWd1", "mWg1", "mWd2", "mWg2")}

    def b1(v):
        return np.ascontiguousarray(
            np.asarray(v, f32).reshape(MC, 128).T)

    def b2(v):
        return np.ascontiguousarray(np.asarray(v, f32).reshape(128, 1))

    common = {
        "wd1": w["Wd1"], "wg1": w["Wg1"], "wmd1": w["mWd1"], "wmg1": w["mWg1"],
        "wd2": w["Wd2"], "wg2": w["Wg2"], "wmd2": w["mWd2"], "wmg2": w["mWg2"],
        "bd1": b1(inputs["bd1"]), "bg1": b1(inputs["bg1"]),
        "mbd1": b1(inputs["mbd1"]), "mbg1": b1(inputs["mbg1"]),
        "bd2": b2(inputs["bd2"]), "bg2": b2(inputs["bg2"]),
        "mbd2": b2(inputs["mbd2"]), "mbg2": b2(inputs["mbg2"]),
        "eye": np.eye(64, dtype=f32),
        "addv": (HW * np.arange(BL, dtype=f32)).reshape(BL, 1),
    }
    fq = np.asarray(inputs["feat_q"], f32).reshape(B, HW, C)
    fk = np.asarray(inputs["feat_k"], f32).reshape(B, HW, C)
    in_maps = []
    for r in range(N_CORES):
        sl = slice(r * BL, (r + 1) * BL)
        m = dict(common)
        m["xqT"] = np.ascontiguousarray(
            fq[sl].reshape(PIX, C).T.astype(bf))
        m["xkT"] = np.ascontiguousarray(
            fk[sl].reshape(PIX, C).T.astype(bf))
        in_maps.append(m)
    return in_maps


def _combine(results):
    sld = smd = slg = spg = 0.0
    for r in range(N_CORES):
        p = np.asarray(results[r]["partials"], np.float64).reshape(-1)
        sld += p[0]
        smd += p[1]
        slg += p[2]
        spg += p[3]
    l_d = (sld - TAU_INV * smd) / GPIX
    l_g = (slg - TAU_INV * spg) / B
    return np.float32(0.5 * l_g + 0.5 * l_d)


def kernel(**inputs) -> np.ndarray:
    nc = _get_nc()
    in_maps = _prep_inputs(inputs)
    res = run_bass_kernel_spmd(nc, in_maps, list(range(N_CORES)))
    return np.asarray(_combine(res.results))


if __name__ == "__main__":
    import reference

    inputs = {k: np.asarray(v) for k, v in reference.setup_inputs().items()}
    got = kernel(**inputs)
    exp = np.asarray(reference.reference(**reference.setup_inputs()))
    print("got", got, "exp", exp, "relerr", abs(got / exp - 1.0))
